# revision 1
# baseline (speedup 1.0000x reference)
"""v6b: v5 + fp16 x32/out IO.

PE in 64x64 mode = 4 independent tiles T0/T2/T8/T10: (row_grp, col_grp) in
{(0,0),(0,64),(64,0),(64,64)}; four spatial tiles stream concurrently, full
128x128 array utilization at K=64, M=64. All matmuls uniform (64,64) tile
size (no mode switches). Moving planes + weights duplicated into both SBUF
partition halves (rows quadrant must match the moving/stationary source).

Per timestep: conv1 27 taps x 8 spatial tiles = 54 pass-slots, conv2 same.
Quantized planes: single duplicated padded tile per plane (qdup), written by
vector-engine copies (gpsimd tensor_copy measured 1.5us -- never use it).

Epilogue algebra (fewer DVE ops):
  conv1: r1=Relu(s1*ps+b1*s1) [act]; m1=min(r1+M, M+127) [DVE];
         q16=m1-M [act, fp16 out]
  conv2: a2=Relu(s2*ps + b2*s2+127) [act]; uy=min(a2+(M-127), M+127) [DVE]
         ax=Relu(s2*x + 127) [act];        yx=min(ax-M, -M+254) [DVE]
         z=uy+yx [gpsimd]; out=Relu(inv_s2*z - 127*inv_s2) [act]
  (uy = M + clip(round(res*s2)); yx = -M+127 + clip(round(x*s2)))
"""

import numpy as np
import concourse.mybir as mybir
from concourse import bacc
from concourse.tile import TileContext
from concourse.bass_utils import run_bass_kernel_spmd

F16 = mybir.dt.float16
F32 = mybir.dt.float32

MANTISA_BIT = 8.0
MAGIC = 12582912.0

N, C, T, H, W = 8, 64, 16, 56, 56
TP, HP, WP = T + 2, H + 2, W + 2
PLANE = HP * WP
SLICE = H * W
ROWS = 7
NT = ROWS * W  # 392
NTILES = H // ROWS  # 8
NG = NTILES // 4  # 2 groups of 4 spatial tiles

_COMPILED = None

TPOS = [(0, 0), (0, 64), (64, 0), (64, 64)]


def _border_memset(nc, tile):
    v = tile[:].rearrange("p (h w) -> p h w", w=WP)
    nc.gpsimd.memset(v[:, 0, :], 0.0)
    nc.gpsimd.memset(v[:, HP - 1, :], 0.0)
    nc.gpsimd.memset(v[:, 1 : HP - 1, 0], 0.0)
    nc.gpsimd.memset(v[:, 1 : HP - 1, WP - 1], 0.0)


def _build():
    nc = bacc.Bacc()
    xpad_d = nc.declare_dram_parameter("xpad", [C, TP, PLANE], F16, isOutput=False)
    x32_d = nc.declare_dram_parameter("x32", [128, T, SLICE // 2], F16, isOutput=False)
    w1_d = nc.declare_dram_parameter("w1p", [128, 27 * 64], F16, isOutput=False)
    w2_d = nc.declare_dram_parameter("w2p", [128, 27 * 64], F16, isOutput=False)
    coeff_d = nc.declare_dram_parameter("coeff", [128, 8], F32, isOutput=False)
    out_d = nc.declare_dram_parameter("out", [C, T * SLICE], F16, isOutput=True)

    def pview(ap):
        return ap.rearrange("p (h w) -> p h w", w=WP)

    with TileContext(nc) as tc:
        with (
            tc.tile_pool(name="big", bufs=1) as bigpool,
            tc.tile_pool(name="xd", bufs=5) as xpool,
            tc.tile_pool(name="qd", bufs=4) as qpool,
            tc.tile_pool(name="x3", bufs=2) as x3pool,
            tc.tile_pool(name="small", bufs=4) as spool,
            tc.tile_pool(name="ps1", bufs=4, space="PSUM") as ps1pool,
            tc.tile_pool(name="ps2", bufs=4, space="PSUM") as ps2pool,
        ):
            w1 = bigpool.tile([128, 27 * 64], F16, tag="w1")
            nc.sync.dma_start(out=w1[:], in_=w1_d[:])
            w2 = bigpool.tile([128, 27 * 64], F16, tag="w2")
            nc.sync.dma_start(out=w2[:], in_=w2_d[:])
            coeff = bigpool.tile([128, 8], F32, tag="coeff")
            nc.sync.dma_start(out=coeff[:], in_=coeff_d[:])

            s1 = coeff[:, 0:1]
            b1s1 = coeff[:, 1:2]
            s2 = coeff[:, 2:3]
            b2s2p = coeff[:, 3:4]  # b2*s2 + 127
            inv_s2 = coeff[:, 4:5]
            bout = coeff[:, 5:6]  # -127*inv_s2
            negM = coeff[:, 6:7]  # -MAGIC
            c127 = coeff[:, 7:8]  # 127.0

            xdup = {}

            def load_x(s):
                # xdup[s]: both halves = x_pad plane s
                xt_ = xpool.tile([128, PLANE], F16, tag="xdup")
                nc.sync.dma_start(out=xt_[0:64, :], in_=xpad_d[:, s, :])
                nc.sync.dma_start(out=xt_[64:128, :], in_=xpad_d[:, s, :])
                xdup[s] = xt_

            for s in range(3):
                load_x(s)

            # qdup[k]: quantized plane k-1 in both halves (padded); qdup[0]=0
            qdup = {}
            qd0_ = qpool.tile([128, PLANE], F16, tag="qdup")
            nc.gpsimd.memset(qd0_[:], 0.0)
            qdup[0] = qd0_

            def mm_group(wtile, ps_pair, planes, g, n_taps):
                # 4 spatial tiles j=4g..4g+3 on PE tiles T0,T2,T8,T10
                for i in range(n_taps):
                    kd, kh, kw = i // 9, (i // 3) % 3, i % 3
                    pv = planes[kd]
                    wsl_lo = wtile[0:64, 64 * i : 64 * i + 64]
                    wsl_hi = wtile[64:128, 64 * i : 64 * i + 64]
                    for q in range(4):
                        r0 = (4 * g + q) * ROWS
                        half = q // 2  # 0: SBUF partitions 0-63, 1: 64-127
                        ps = ps_pair[half]
                        out_ap = ps[0:64, :] if q % 2 == 0 else ps[64:128, :]
                        mv = pv[64 * half : 64 * half + 64,
                                r0 + kh : r0 + kh + ROWS, kw : kw + W]
                        nc.tensor.matmul(
                            out_ap,
                            wsl_hi if half else wsl_lo,
                            mv,
                            start=(i == 0), stop=(i == n_taps - 1),
                            tile_position=TPOS[q],
                            skip_group_check=True,
                        )

            for t in range(T + 1):
                if t < T:
                    if t + 3 <= TP - 1:
                        load_x(t + 3)
                    qd_ = qpool.tile([128, PLANE], F16, tag="qdup")
                    _border_memset(nc, qd_)
                    qdup[t + 1] = qd_
                    qn_v = pview(qdup[t + 1][:])
                    planes1 = [pview(xdup[t + kd][:]) for kd in range(3)]
                    for g in range(NG):
                        psA = ps1pool.tile([128, NT], F32, tag="ps1")
                        psB = ps1pool.tile([128, NT], F32, tag="ps1")
                        mm_group(w1, (psA, psB), planes1, g, 27)
                        for b, ps in enumerate((psA, psB)):
                            r1 = spool.tile([128, NT], F32, tag="r1")
                            nc.scalar.activation(
                                r1[:], ps[:], mybir.ActivationFunctionType.Relu,
                                bias=b1s1, scale=s1,
                            )
                            m1 = spool.tile([128, NT], F32, tag="m1")
                            nc.vector.tensor_scalar(
                                out=m1[:], in0=r1[:],
                                scalar1=MAGIC, scalar2=MAGIC + 127.0,
                                op0=mybir.AluOpType.add, op1=mybir.AluOpType.min,
                            )
                            q16 = spool.tile([128, NT], F16, tag="q16")
                            nc.scalar.activation(
                                q16[:], m1[:], mybir.ActivationFunctionType.Identity,
                                bias=negM, scale=1.0,
                            )
                            # placements: plane t interior rows of tiles
                            # j = 4g+2b (q16 low) and 4g+2b+1 (q16 high)
                            for h in range(2):
                                qv = q16[64 * h : 64 * h + 64, :].rearrange(
                                    "p (r w) -> p r w", w=W
                                )
                                rr = 1 + (4 * g + 2 * b + h) * ROWS
                                nc.vector.tensor_copy(
                                    qn_v[0:64, rr : rr + ROWS, 1 : 1 + W], qv
                                )
                                nc.vector.tensor_copy(
                                    qn_v[64:128, rr : rr + ROWS, 1 : 1 + W], qv
                                )

                if t >= 1:
                    u = t - 1
                    x32 = x3pool.tile([128, SLICE // 2], F16, tag="x32")
                    nc.sync.dma_start(out=x32[:], in_=x32_d[:, u, :])
                    n_taps = 27 if u + 2 <= T else 18
                    planes2 = [pview(qdup[u + kd][:]) for kd in range(3 if n_taps == 27 else 2)]
                    if n_taps == 18:
                        planes2.append(None)
                    for g in range(NG):
                        psA = ps2pool.tile([128, NT], F32, tag="ps2")
                        psB = ps2pool.tile([128, NT], F32, tag="ps2")
                        mm_group(w2, (psA, psB), planes2, g, n_taps)
                        for b, ps in enumerate((psA, psB)):
                            p2 = 2 * g + b  # pair index: tiles 4g+2b, 4g+2b+1
                            a2 = spool.tile([128, NT], F32, tag="a2")
                            nc.scalar.activation(
                                a2[:], ps[:], mybir.ActivationFunctionType.Relu,
                                bias=b2s2p, scale=s2,
                            )
                            uy = spool.tile([128, NT], F32, tag="uy")
                            nc.vector.tensor_scalar(
                                out=uy[:], in0=a2[:],
                                scalar1=MAGIC - 127.0, scalar2=MAGIC + 127.0,
                                op0=mybir.AluOpType.add, op1=mybir.AluOpType.min,
                            )
                            ax = spool.tile([128, NT], F32, tag="ax")
                            nc.scalar.activation(
                                ax[:], x32[:, p2 * NT : (p2 + 1) * NT],
                                mybir.ActivationFunctionType.Relu,
                                bias=c127, scale=s2,
                            )
                            yx = spool.tile([128, NT], F32, tag="yx")
                            nc.vector.tensor_scalar(
                                out=yx[:], in0=ax[:],
                                scalar1=-MAGIC, scalar2=-MAGIC + 254.0,
                                op0=mybir.AluOpType.add, op1=mybir.AluOpType.min,
                            )
                            z = spool.tile([128, NT], F32, tag="z")
                            nc.gpsimd.tensor_add(z[:], uy[:], yx[:])
                            o_sb = spool.tile([128, NT], F16, tag="osb")
                            nc.scalar.activation(
                                o_sb[:], z[:], mybir.ActivationFunctionType.Relu,
                                bias=bout, scale=inv_s2,
                            )
                            offA = u * SLICE + (4 * g + 2 * b) * NT
                            offB = u * SLICE + (4 * g + 2 * b + 1) * NT
                            nc.sync.dma_start(
                                out=out_d[:, offA : offA + NT], in_=o_sb[0:64, :]
                            )
                            nc.sync.dma_start(
                                out=out_d[:, offB : offB + NT], in_=o_sb[64:128, :]
                            )
    nc.compile()
    return nc


def _host_pack(x, w1, b1, w2, b2, exp1, exp2):
    scale1 = np.exp2(MANTISA_BIT - 1.0 - exp1).astype(np.float32)
    scale2 = np.exp2(MANTISA_BIT - 1.0 - exp2).astype(np.float32)

    def pack_w(wt):
        # wt: [kd,kh,kw,i,o] fp32 -> [128, 27*64] fp16, dup along partitions
        p = wt.reshape(27, 64, 64)
        p = np.ascontiguousarray(np.transpose(p, (1, 0, 2))).reshape(64, 27 * 64)
        return np.concatenate([p, p], axis=0).astype(np.float16)

    w1t = np.transpose(w1, (2, 3, 4, 1, 0)).astype(np.float32)
    w1p = pack_w(w1t)
    w2f = (w2 / scale1[None, :, None, None, None]).astype(np.float32)
    w2t = np.transpose(w2f, (2, 3, 4, 1, 0)).astype(np.float32)
    w2p = pack_w(w2t)

    c64 = np.zeros((64, 8), dtype=np.float32)
    c64[:, 0] = scale1
    c64[:, 1] = b1 * scale1
    c64[:, 2] = scale2
    c64[:, 3] = b2 * scale2 + 127.0
    c64[:, 4] = 1.0 / scale2
    c64[:, 5] = -127.0 / scale2
    c64[:, 6] = -MAGIC
    c64[:, 7] = 127.0
    coeff = np.concatenate([c64, c64], axis=0)

    shared = {"w1p": w1p, "w2p": w2p, "coeff": coeff}
    in_maps = []
    for n in range(N):
        xp = np.pad(x[n], ((0, 0), (1, 1), (1, 1), (1, 1))).astype(np.float16)
        m = dict(shared)
        m["xpad"] = np.ascontiguousarray(xp.reshape(C, TP, PLANE))
        xt = x[n].reshape(C, T, NTILES, NT)
        x32 = np.concatenate([xt[:, :, 0::2, :], xt[:, :, 1::2, :]], axis=0)
        m["x32"] = np.ascontiguousarray(
            x32.reshape(128, T, SLICE // 2).astype(np.float16)
        )
        in_maps.append(m)
    return in_maps


def kernel(x, w1, b1, w2, b2, exp1, exp2):
    global _COMPILED
    x = np.asarray(x, dtype=np.float32)
    w1 = np.asarray(w1, dtype=np.float32)
    b1 = np.asarray(b1, dtype=np.float32)
    w2 = np.asarray(w2, dtype=np.float32)
    b2 = np.asarray(b2, dtype=np.float32)
    exp1 = np.asarray(exp1, dtype=np.float32)
    exp2 = np.asarray(exp2, dtype=np.float32)
    if _COMPILED is None:
        _COMPILED = _build()
    in_maps = _host_pack(x, w1, b1, w2, b2, exp1, exp2)
    res = run_bass_kernel_spmd(_COMPILED, in_maps, core_ids=list(range(N)))
    out = np.stack([np.asarray(res.results[i]["out"], dtype=np.float32).reshape(C, T, H, W) for i in range(N)])
    return out.astype(np.float32)



# revision 2
# speedup vs baseline: 1.0385x; 1.0385x over previous
"""v7: restructured epilogues + split-plane layout + offset-1536 BFP quant.

Same PE structure as v6b (64x64 quad tiles, 27-tap accumulation, fp16) but:

- Quantization via fp16-convert rounding at +1536/+1663 offsets:
    conv1: v = Relu(s1*ps + b1*s1) [ACT]; q = min(v,127)+1536 -> fp16 [DVE]
           (fp16 rounds to integer grid on [1024,2048))
    conv2: A = s2*ps + badj [ACT]; A~ = clamp(A,1536,1790) -> fp16 [DVE]
           z = A~ + xq [DVE TT]   (xq = host-quantized clip(round(s2*x)) - 1663)
           out = max(z,0)*inv_s2 -> fp16 [DVE]
  The +1536 activation offset flows through conv2's matmul; it is removed by a
  per-channel bias correction badj = b2*s2 + 1663 - s2*1536*sum(w2f) computed on
  host against the fp16-rounded weights. Plane borders are memset to 1536
  (= value 0 in offset space) so the correction is position-independent.

- Split-plane SBUF layout [128, 32, 58]: partition half0 stores padded plane
  rows [0..15]+[28..43], half1 rows [14..29]+[42..57]. Each PE quadrant reads
  moving data from its own half; spatial tiles are assigned so 6 (not 16)
  interior writes + 6 single-row halo writes build each quantized plane.
  x input planes use the same layout packed on host (halves the x DMA bytes).

- conv taps ordered kd-major with groups interleaved: conv2's kd=2 taps (which
  need the plane conv1 just produced) sit behind ~6us of kd=0/1 PE work.

- Output staged per timestep into [128, 4*392] SBUF and DMA'd contiguously.
"""

import numpy as np
import concourse.mybir as mybir
from concourse import bacc
from concourse.tile import TileContext
from concourse.bass_utils import run_bass_kernel_spmd

F16 = mybir.dt.float16
F32 = mybir.dt.float32

N, C, T, H, W = 8, 64, 16, 56, 56
TP = T + 2
SH, SW = 32, 58        # split-plane storage rows / cols
PLANE = SH * SW        # 1856 per half
ROWS = 7
NT = ROWS * W          # 392
NTAP = 27

# tile j on (group g, quadrant q): j = 4g + SIG[q]; SIG maps quadrant->tile idx
TPOS = [(0, 0), (0, 64), (64, 0), (64, 64)]
SIG = [0, 1, 3, 2]
# storage-row start (within half, before +16g) per quadrant
BS = [0, 7, 7, 0]
# psA holds q0 (part 0:64) + q3 (64:128); psB holds q2 (0:64) + q1 (64:128)
# block b = 2g + s (s=0:A, 1:B); tiles: p<64 -> TA[b], p>=64 -> TB[b]
TA = [0, 3, 4, 7]
TB = [2, 1, 6, 5]

# halo single-row writes per conv1 output plane:
# (g, src_ps 'A'/'B', src_part_half, src_row, dst_half, dst_strow)
HALO = [
    (0, 'A', 1, 0, 0, 15),   # plane row 15 = tile2 row0 -> half0 st15
    (0, 'B', 1, 6, 1, 0),    # plane row 14 = tile1 row6 -> half1 st0
    (0, 'B', 0, 6, 0, 16),   # plane row 28 = tile3 row6 -> half0 st16
    (1, 'A', 0, 0, 1, 15),   # plane row 29 = tile4 row0 -> half1 st15
    (1, 'A', 1, 0, 0, 31),   # plane row 43 = tile6 row0 -> half0 st31
    (1, 'B', 1, 6, 1, 16),   # plane row 42 = tile5 row6 -> half1 st16
]

_COMPILED = None


def _build():
    nc = bacc.Bacc()
    xpad_d = nc.declare_dram_parameter("xpad", [128, TP, PLANE], F16, isOutput=False)
    xq_d = nc.declare_dram_parameter("xq", [128, T, 4 * NT], F16, isOutput=False)
    w1_d = nc.declare_dram_parameter("w1p", [128, NTAP * 64], F16, isOutput=False)
    w2_d = nc.declare_dram_parameter("w2p", [128, NTAP * 64], F16, isOutput=False)
    coeff_d = nc.declare_dram_parameter("coeff", [128, 6], F32, isOutput=False)
    out_d = nc.declare_dram_parameter("out", [128, T * 4 * NT], F16, isOutput=True)

    def sview(ap):
        return ap.rearrange("p (r c) -> p r c", c=SW)

    with TileContext(nc) as tc:
        with (
            tc.tile_pool(name="big", bufs=1) as bigpool,
            tc.tile_pool(name="xd", bufs=5) as xpool,
            tc.tile_pool(name="qd", bufs=4) as qpool,
            tc.tile_pool(name="xq", bufs=3) as xqpool,
            tc.tile_pool(name="v", bufs=4) as vpool,
            tc.tile_pool(name="a", bufs=4) as apool,
            tc.tile_pool(name="at", bufs=4) as atpool,
            tc.tile_pool(name="z", bufs=4) as zpool,
            tc.tile_pool(name="os", bufs=2) as opool,
            tc.tile_pool(name="ps1", bufs=4, space="PSUM") as ps1pool,
            tc.tile_pool(name="ps2", bufs=4, space="PSUM") as ps2pool,
        ):
            xpl = {}

            def load_x(s):
                xt_ = xpool.tile([128, PLANE], F16, tag="xpl")
                nc.sync.dma_start(out=xt_[:], in_=xpad_d[:, s, :])
                xpl[s] = xt_

            load_x(0)
            w1 = bigpool.tile([128, NTAP * 64], F16, tag="w1")
            nc.sync.dma_start(out=w1[:], in_=w1_d[:])
            coeff = bigpool.tile([128, 6], F32, tag="coeff")
            nc.sync.dma_start(out=coeff[:], in_=coeff_d[:])
            load_x(1)
            load_x(2)
            w2 = bigpool.tile([128, NTAP * 64], F16, tag="w2")
            nc.sync.dma_start(out=w2[:], in_=w2_d[:])

            s1 = coeff[:, 0:1]
            b1s1 = coeff[:, 1:2]
            s2 = coeff[:, 2:3]
            badj27 = coeff[:, 3:4]
            badj18 = coeff[:, 4:5]
            inv_s2 = coeff[:, 5:6]

            # qdup[k] = quantized conv1 output plane k-1 (offset +1536);
            # qdup[0] = all-1536 (temporal zero pad)
            qdup = {}
            qd0_ = qpool.tile([128, PLANE], F16, tag="qdup")
            nc.gpsimd.memset(qd0_[:], 1536.0)
            qdup[0] = qd0_

            def plane_border(qp):
                v = sview(qp[:])
                nc.gpsimd.memset(v[0:64, 0, :], 1536.0)
                nc.gpsimd.memset(v[64:128, SH - 1, :], 1536.0)
                nc.gpsimd.memset(v[:, :, 0], 1536.0)
                nc.gpsimd.memset(v[:, :, SW - 1], 1536.0)

            def conv_mms(wt, planes, ps, n_taps):
                # kd-major, groups interleaved; 4 quadrants per (tap, g)
                for kd in range(n_taps // 9):
                    pv = planes[kd]
                    for g in range(2):
                        psA, psB = ps[g]
                        for kh in range(3):
                            for kw in range(3):
                                i = kd * 9 + kh * 3 + kw
                                for q in range(4):
                                    hf = q // 2
                                    wsl = wt[64 * hf: 64 * hf + 64,
                                             64 * i: 64 * i + 64]
                                    mv = pv[64 * hf: 64 * hf + 64,
                                            16 * g + BS[q] + kh:
                                            16 * g + BS[q] + kh + ROWS,
                                            kw: kw + W]
                                    pst = psA if q in (0, 3) else psB
                                    out_ap = (pst[0:64, :] if q in (0, 2)
                                              else pst[64:128, :])
                                    nc.tensor.matmul(
                                        out_ap, wsl, mv,
                                        start=(i == 0), stop=(i == n_taps - 1),
                                        tile_position=TPOS[q],
                                        skip_group_check=True,
                                    )

            for t in range(T + 1):
                if t < T:
                    if t + 3 <= TP - 1:
                        load_x(t + 3)
                    qp = qpool.tile([128, PLANE], F16, tag="qdup")
                    plane_border(qp)
                    qdup[t + 1] = qp
                    qv = sview(qp[:])

                    planes1 = [sview(xpl[t + kd][:]) for kd in range(3)]
                    ps1 = []
                    for g in range(2):
                        psA = ps1pool.tile([128, NT], F32, tag="ps1")
                        psB = ps1pool.tile([128, NT], F32, tag="ps1")
                        ps1.append((psA, psB))
                    conv_mms(w1, planes1, ps1, NTAP)

                    vs = {}
                    for g in range(2):
                        psA, psB = ps1[g]
                        vA = vpool.tile([128, NT], F32, tag="v")
                        nc.scalar.activation(
                            vA[:], psA[:], mybir.ActivationFunctionType.Relu,
                            bias=b1s1, scale=s1,
                        )
                        vB = vpool.tile([128, NT], F32, tag="v")
                        nc.scalar.activation(
                            vB[:], psB[:], mybir.ActivationFunctionType.Relu,
                            bias=b1s1, scale=s1,
                        )
                        vs[(g, 'A')] = vA
                        vs[(g, 'B')] = vB
                        vAv = vA[:].rearrange("p (r w) -> p r w", w=W)
                        vBv = vB[:].rearrange("p (r w) -> p r w", w=W)
                        # psA: both halves land at same storage rows
                        nc.vector.tensor_scalar(
                            out=qv[:, 16 * g + 1: 16 * g + 8, 1: 1 + W],
                            in0=vAv, scalar1=127.0, scalar2=1536.0,
                            op0=mybir.AluOpType.min, op1=mybir.AluOpType.add,
                        )
                        # psB: crossed halves
                        nc.vector.tensor_scalar(
                            out=qv[64:128, 16 * g + 8: 16 * g + 15, 1: 1 + W],
                            in0=vBv[0:64], scalar1=127.0, scalar2=1536.0,
                            op0=mybir.AluOpType.min, op1=mybir.AluOpType.add,
                        )
                        nc.vector.tensor_scalar(
                            out=qv[0:64, 16 * g + 8: 16 * g + 15, 1: 1 + W],
                            in0=vBv[64:128], scalar1=127.0, scalar2=1536.0,
                            op0=mybir.AluOpType.min, op1=mybir.AluOpType.add,
                        )
                    for (g, sp, sh, srow, dh, drow) in HALO:
                        sv = vs[(g, sp)][:].rearrange("p (r w) -> p r w", w=W)
                        nc.vector.tensor_scalar(
                            out=qv[64 * dh: 64 * dh + 64, drow: drow + 1, 1: 1 + W],
                            in0=sv[64 * sh: 64 * sh + 64, srow: srow + 1, :],
                            scalar1=127.0, scalar2=1536.0,
                            op0=mybir.AluOpType.min, op1=mybir.AluOpType.add,
                        )

                    # prefetch xq for conv2(t) used next iteration
                    xqt = xqpool.tile([128, 4 * NT], F16, tag="xq")
                    nc.sync.dma_start(out=xqt[:], in_=xq_d[:, t, :])
                    if t == 0:
                        xq_tiles = {}
                    xq_tiles[t] = xqt

                if t >= 1:
                    u = t - 1
                    n_taps = NTAP if u + 2 <= T else 18
                    planes2 = [sview(qdup[u + kd][:])
                               for kd in range(n_taps // 9)]
                    ps2 = []
                    for g in range(2):
                        psA = ps2pool.tile([128, NT], F32, tag="ps2")
                        psB = ps2pool.tile([128, NT], F32, tag="ps2")
                        ps2.append((psA, psB))
                    conv_mms(w2, planes2, ps2, n_taps)

                    badj = badj27 if n_taps == NTAP else badj18
                    xqt = xq_tiles.pop(u)
                    ostage = opool.tile([128, 4 * NT], F16, tag="os")
                    for g in range(2):
                        for s, pst in enumerate(ps2[g]):
                            b = 2 * g + s
                            A = apool.tile([128, NT], F32, tag="A")
                            nc.scalar.activation(
                                A[:], pst[:],
                                mybir.ActivationFunctionType.Identity,
                                bias=badj, scale=s2,
                            )
                            At = atpool.tile([128, NT], F16, tag="At")
                            nc.vector.tensor_scalar(
                                out=At[:], in0=A[:],
                                scalar1=1536.0, scalar2=1790.0,
                                op0=mybir.AluOpType.max, op1=mybir.AluOpType.min,
                            )
                            z = zpool.tile([128, NT], F16, tag="z")
                            nc.vector.tensor_tensor(
                                out=z[:], in0=At[:],
                                in1=xqt[:, b * NT: (b + 1) * NT],
                                op=mybir.AluOpType.add,
                            )
                            nc.vector.tensor_scalar(
                                out=ostage[:, b * NT: (b + 1) * NT], in0=z[:],
                                scalar1=0.0, scalar2=inv_s2,
                                op0=mybir.AluOpType.max, op1=mybir.AluOpType.mult,
                            )
                    nc.sync.dma_start(
                        out=out_d[:, u * 4 * NT: (u + 1) * 4 * NT],
                        in_=ostage[:],
                    )
    nc.compile()
    return nc


def _host_pack(x, w1, b1, w2, b2, exp1, exp2):
    scale1 = np.exp2(7.0 - exp1.astype(np.float64))
    scale2 = np.exp2(7.0 - exp2.astype(np.float64))

    def pack_w(wt):
        # wt: [kd,kh,kw,i,o] -> [128, 27*64] fp16 (dup halves)
        p = wt.reshape(NTAP, 64, 64)
        p = np.ascontiguousarray(np.transpose(p, (1, 0, 2))).reshape(64, NTAP * 64)
        p16 = p.astype(np.float16)
        return np.concatenate([p16, p16], axis=0)

    w1t = np.transpose(w1, (2, 3, 4, 1, 0)).astype(np.float32)
    w1p = pack_w(w1t)
    w2f = (w2.astype(np.float64) / scale1[None, :, None, None, None])
    w2t = np.transpose(w2f, (2, 3, 4, 1, 0)).astype(np.float32)
    w2p = pack_w(w2t)

    # offset corrections against fp16-rounded w2f: [kd,kh,kw,i,o]
    w2t16 = w2t.astype(np.float16).astype(np.float64).reshape(3, 9 * 64, 64)
    off27 = 1536.0 * w2t16.sum(axis=(0, 1))
    off18 = 1536.0 * w2t16[:2].sum(axis=(0, 1))

    c64 = np.zeros((64, 6), dtype=np.float64)
    c64[:, 0] = scale1
    c64[:, 1] = b1.astype(np.float64) * scale1
    c64[:, 2] = scale2
    c64[:, 3] = b2.astype(np.float64) * scale2 + 1663.0 - scale2 * off27
    c64[:, 4] = b2.astype(np.float64) * scale2 + 1663.0 - scale2 * off18
    c64[:, 5] = 1.0 / scale2
    coeff = np.concatenate([c64, c64], axis=0).astype(np.float32)

    idx0 = list(range(0, 16)) + list(range(28, 44))
    idx1 = list(range(14, 30)) + list(range(42, 58))

    shared = {"w1p": w1p, "w2p": w2p, "coeff": coeff}
    in_maps = []
    for n in range(N):
        xp = np.pad(x[n], ((0, 0), (1, 1), (1, 1), (1, 1))).astype(np.float16)
        xs = np.stack([xp[:, :, idx0, :], xp[:, :, idx1, :]], axis=0)
        m = dict(shared)
        m["xpad"] = np.ascontiguousarray(xs.reshape(128, TP, PLANE))

        cX = np.clip(np.round(x[n].astype(np.float64)
                              * scale2[:, None, None, None]), -127, 127)
        xq = (cX - 1663.0).astype(np.float16).reshape(C, T, 8, ROWS, W)
        xqp = np.stack([xq[:, :, TA], xq[:, :, TB]], axis=0)
        m["xq"] = np.ascontiguousarray(xqp.reshape(128, T, 4 * NT))
        in_maps.append(m)
    return in_maps


def kernel(x, w1, b1, w2, b2, exp1, exp2):
    global _COMPILED
    x = np.asarray(x, dtype=np.float32)
    w1 = np.asarray(w1, dtype=np.float32)
    b1 = np.asarray(b1, dtype=np.float32)
    w2 = np.asarray(w2, dtype=np.float32)
    b2 = np.asarray(b2, dtype=np.float32)
    exp1 = np.asarray(exp1, dtype=np.float32)
    exp2 = np.asarray(exp2, dtype=np.float32)
    if _COMPILED is None:
        _COMPILED = _build()
    in_maps = _host_pack(x, w1, b1, w2, b2, exp1, exp2)
    res = run_bass_kernel_spmd(_COMPILED, in_maps, core_ids=list(range(N)))
    out = np.empty((N, C, T, H, W), dtype=np.float32)
    for n in range(N):
        od = np.asarray(res.results[n]["out"], dtype=np.float32)
        od = od.reshape(2, 64, T, 4, ROWS, W)
        full = np.empty((C, T, 8, ROWS, W), dtype=np.float32)
        for half, tbl in ((0, TA), (1, TB)):
            for b in range(4):
                full[:, :, tbl[b]] = od[half, :, :, b]
        out[n] = full.reshape(C, T, H, W)
    return out
